# revision 1
# baseline (speedup 1.0000x reference)
"""Trainium2 Bass kernel for single-head causal attention.

Problem: B=4, S=2048, E=1024, H=64 fp32.
  q = x@Wq+bq; k = x@Wk+bk; v = x@Wv+bv
  out = softmax(causal(q k^T / sqrt(H))) v

Sharding: 8 cores; core c = (batch b=c//2, query-half h=c%2).
Each core computes full K/V for its batch but attention for only its
1024 queries (h=0: front 256 of each 512-tile, h=1: back 256).
SPMD-uniform: the per-core query selection is realized by a host-side
128-column block permutation of x^T (h=1 swaps the half-tiles within
each 512 tile), and causality by per-core mask tables; the device
program is identical on all cores.

All matmul operands are bf16 (1 cycle/col on the PE vs 4 for fp32),
fp32 accumulation in PSUM. x is transposed and cast to bf16 on the
host: no PE transposes of x, and DMA traffic halves (4MB/core).
The device returns pv^T tiles [65, 256] per q-tile (row 64 = softmax
denominator from a ones-column in V); the host does the final
divide + transpose, removing the whole output transpose stage.

Engine assignment: PE = projections + scores + PV + V transposes
(plus a HAM-warmup matmul burst during the initial DMA wait);
ACT = exp (512-wide pairs) + Q copy; DVE = K/V copies, masks, V-block
copies, pv copies, bias cast. K^T/Q^T live on partitions 64:127
([Wv|Wk] packing) so every PSUM->SBUF copy is partition-aligned.
(gpsimd cannot touch PSUM on HW, so it only does memsets.)

The kernel is one software-pipelined stream over 20 score/PV "pairs"
(2 k-blocks each); rounds 2 and 3 are interleaved pair-by-pair so the
exp (ACT) load of round 3 overlaps PE-heavy projection work, and an
unmasked pair closes round 3 so the final exp->mask->PV tail is short.
Projection closures for future rounds (split into chunk-gated groups
A/K1/K2/T placed at slots matching the xt DMA arrival order) and
output stages are used as PE filler between pairs, so the in-order PE
queue never stalls on the exp->mask->PV chain (PDEPTH=2 lookahead).
PSUM banks (8): scores 2 + pkv 1 + ppv 2 + pq 1 + V-trans 2.
"""

import sys
from collections import deque
from contextlib import ExitStack

import numpy as np
import ml_dtypes

if "/opt/trn_rl_repo" not in sys.path:
    sys.path.insert(0, "/opt/trn_rl_repo")

import concourse.bacc as bacc
import concourse.mybir as mybir
import concourse.tile as tile

B, S, E, H = 4, 2048, 1024, 64
NCORES = 8
F32 = mybir.dt.float32
BF16 = mybir.dt.bfloat16
AF = mybir.ActivationFunctionType
NPBF16 = ml_dtypes.bfloat16

ST = 512          # s-tile width (projections / one round)
NST = S // ST     # 4 rounds
NEC = E // 128    # 8 e-chunks (contraction)
QW = 256          # per-core q-tile width in attention
PW = 2 * QW       # paired width (2 k-blocks per exp)
XCH = 256         # xt DMA chunk width
PDEPTH = 2        # attention software-pipeline depth, in pairs

# wpack column offsets (DMA-split points: wq+identb | wkv | mask)
W_Q = 0
W_ID = W_Q + NEC * H
W_BVK = W_ID + 128
W_BQ = W_BVK + 1
W_KV = W_BQ + 1
W_MASK = W_KV + NEC * 128
W_COLS = W_MASK + 4 * QW

# rounds 2/3 interleaved (round-3 exp load overlaps round-2+proj PE work),
# round 3 starting after its Q-pass chunk (x6); the final pairs are
# ordered so an unmasked pair closes round 3 (shorter tail chain).
PAIR_ORDER = [(0, 0), (0, 1),
              (1, 0), (1, 1), (1, 2), (1, 3),
              (2, 0), (2, 1), (2, 2), (3, 0), (2, 3), (3, 1), (2, 4),
              (3, 2), (2, 5), (3, 3), (3, 6), (3, 7), (3, 5), (3, 4)]
# last-emitted pair per round (closes that round's PV accumulation)
LAST_PAIR = {0: 1, 1: 3, 2: 5, 3: 4}


def build_program():
    nc = bacc.Bacc("TRN2", target_bir_lowering=False, debug=False,
                   num_devices=NCORES)

    wp_d = nc.dram_tensor("wpack", [128, W_COLS], BF16, kind="ExternalInput")
    xt_d = nc.dram_tensor("xt", [128, NEC, S], BF16, kind="ExternalInput")
    y_d = nc.dram_tensor("y", [H + 1, NST, QW], F32, kind="ExternalOutput")

    with tile.TileContext(nc) as tc, ExitStack() as ctx:
        singles = ctx.enter_context(tc.tile_pool(name="singles", bufs=1))
        vtpool = ctx.enter_context(tc.tile_pool(name="vtpool", bufs=2))
        ppool = ctx.enter_context(tc.tile_pool(name="ppool", bufs=4))
        opool = ctx.enter_context(tc.tile_pool(name="opool", bufs=4))
        # PSUM 8 banks: sc-pairs 2 + kv 1 + ppv 2 + pq 1 + vtrans 2
        psA = ctx.enter_context(tc.tile_pool(name="psA", bufs=2, space="PSUM"))
        psKV = ctx.enter_context(tc.tile_pool(name="psKV", bufs=1,
                                              space="PSUM"))
        psB = ctx.enter_context(tc.tile_pool(name="psB", bufs=2, space="PSUM"))
        psQ = ctx.enter_context(tc.tile_pool(name="psQ", bufs=1, space="PSUM"))
        psC = ctx.enter_context(tc.tile_pool(name="psC", bufs=2, space="PSUM"))

        # ---- SBUF tiles ----
        wp = singles.tile([128, W_COLS], BF16)
        xt = singles.tile([128, NEC, S], BF16)

        # ---- DMAs, in dependency-priority order ----
        def xt_dma(xc):
            nc.sync.dma_start(out=xt[:, :, xc * XCH:(xc + 1) * XCH],
                              in_=xt_d[:, :, xc * XCH:(xc + 1) * XCH])

        nc.sync.dma_start(out=wp[:, W_Q:W_KV], in_=wp_d[:, W_Q:W_KV])
        for sub in range(2):  # xt chunk 0 split by e-chunk halves
            nc.sync.dma_start(out=xt[:, sub * 4:(sub + 1) * 4, 0:XCH],
                              in_=xt_d[:, sub * 4:(sub + 1) * 4, 0:XCH])
        nc.sync.dma_start(out=wp[:, W_KV:W_MASK], in_=wp_d[:, W_KV:W_MASK])
        xt_dma(1)
        xt_dma(2)
        nc.sync.dma_start(out=wp[:, W_MASK:W_MASK + PW],
                          in_=wp_d[:, W_MASK:W_MASK + PW])
        xt_dma(3)
        nc.sync.dma_start(out=wp[:, W_MASK + PW:W_COLS],
                          in_=wp_d[:, W_MASK + PW:W_COLS])
        xt_dma(4)
        xt_dma(6)   # round-3 Q-pass chunk before the round-2/3 k-tails
        xt_dma(5)
        xt_dma(7)

        def wkv_ap(ec):   # [Wv | Wk] chunk: out rows 0:64 = V, 64:128 = K
            return wp[:, W_KV + ec * 128: W_KV + (ec + 1) * 128]

        def wq_ap(ec):
            return wp[:, W_Q + ec * H: W_Q + (ec + 1) * H]

        def mask_pair_ap(jj):   # [128, 512] = k-blocks (2jj, 2jj+1)
            return wp[:, W_MASK + jj * PW: W_MASK + (jj + 1) * PW]

        identb = wp[:, W_ID:W_ID + 128]
        # HAM warmup: dummy matmuls on a just-memset tile run while the
        # first DMAs stream, so the PE clock-gate is already released
        # (and the sim's p-state ramp elapsed) when real work arrives.
        warm = singles.tile([128, 256], BF16)
        nc.vector.memset(warm, 0.0)
        for w in range(6):
            pw_ = psA.tile([128, PW], F32, tag="big", name=f"warm_{w}")
            nc.tensor.matmul(pw_[:, 0:256], warm[:, 0:128], warm,
                             start=True, stop=True)
        # biases ride in wpack as bf16; engines need fp32 scalars
        fb = singles.tile([128, 2], F32)
        nc.vector.tensor_copy(fb, wp[:, W_BVK:W_BVK + 2])
        bvk = fb[:, 0:1]        # rows 0:64 = bv, rows 64:128 = bk
        bq = fb[0:H, 1:2]

        qt_all = singles.tile([128, NST, QW], BF16)  # rows 64:128 = own Q^T
        kt_all = singles.tile([128, S], BF16)        # rows 64:128 = K^T
        v_all = singles.tile([128, S // 128, H + 1], BF16)  # V blocks
        nc.vector.memset(v_all[:, :, H:H + 1], 1.0)

        # ---- emission helpers ----
        # Projection work for round t is split into chunk-gated groups so
        # the in-order PE queue never blocks on a not-yet-arrived xt chunk:
        #   A(t)  = Q pass + qt copy            (needs xt chunk 2t)
        #   K1(t) = KV pass cols 0:256 + copies (needs xt chunk 2t)
        #   K2(t) = KV pass cols 256:512        (needs xt chunk 2t+1)
        #   T1/T2(t) = V-block transposes for each half
        proj_states = {}

        def emit_A(t):
            def f():
                st_ = proj_states.setdefault(t, {})
                st_["pq"] = psQ.tile([H, QW], F32, tag="pq",
                                     name=f"pq_{t}")
                for ec in range(NEC):
                    nc.tensor.matmul(st_["pq"], wq_ap(ec),
                                     xt[:, ec, t * ST:t * ST + QW],
                                     start=(ec == 0), stop=(ec == NEC - 1))
                nc.scalar.activation(qt_all[64:128, t, :], st_["pq"],
                                     AF.Identity, bias=bq)
            return [f]

        def emit_K(t, s):
            c0, c1 = s * QW, (s + 1) * QW
            def f():
                st_ = proj_states.setdefault(t, {})
                if s == 0:
                    st_["pkv"] = psKV.tile([128, ST], F32, tag="kv",
                                           name=f"pkv_{t}")
                    st_["vt"] = vtpool.tile([H, ST], BF16, tag="vt",
                                            name=f"vt_{t}")
                pkv = st_["pkv"]
                for ec in range(NEC):
                    nc.tensor.matmul(pkv[:, c0:c1], wkv_ap(ec),
                                     xt[:, ec, t * ST + c0:t * ST + c1],
                                     start=(ec == 0), stop=(ec == NEC - 1))
                nc.vector.tensor_scalar_add(st_["vt"][:, c0:c1],
                                            pkv[0:H, c0:c1], bvk[0:H, :])
                nc.vector.tensor_scalar_add(
                    kt_all[64:128, t * ST + c0:t * ST + c1],
                    pkv[64:128, c0:c1], bvk[64:128, :])
            return [f]

        def emit_T(t, s):
            def f():
                vt = proj_states[t]["vt"]
                pv = psC.tile([128, 2, H], BF16, tag="small",
                              name=f"pvt_{t}_{s}")
                for i in range(2):
                    sb = 2 * s + i
                    nc.tensor.transpose(pv[:, i, :],
                                        vt[:, sb * 128:(sb + 1) * 128],
                                        identb[0:H, 0:H])
                blk = t * 4 + 2 * s
                nc.vector.tensor_copy(v_all[:, blk:blk + 2, 0:H], pv)
            return [f]

        def emit_out(t, ppv):
            """Output closures for round t: copy pv^T out; host divides."""
            state = {}

            def copy():
                state["pv_sb"] = opool.tile([H + 1, QW], F32, tag="pv_sb",
                                            name=f"pvsb_{t}")
                nc.vector.tensor_copy(state["pv_sb"], ppv)

            def dma():
                nc.sync.dma_start(out=y_d[:, t, :], in_=state["pv_sb"])

            return [copy, dma]

        # ---- static filler schedule (slot -> closure groups) ----
        # Placement tracks the xt DMA arrival order:
        # x0,[w],x1,x2,[mask01],x3,[mask23],x4,x6,x5,x7
        schedule = {
            0: emit_K(0, 1) + emit_T(0, 0),
            1: emit_T(0, 1) + emit_A(1),
            2: emit_K(1, 0),
            3: emit_T(1, 0),
            4: emit_K(1, 1) + emit_T(1, 1),
            5: emit_A(2),
            6: emit_K(2, 0),
            7: emit_T(2, 0) + emit_A(3),
            10: emit_K(2, 1),
            11: emit_T(2, 1),
            12: emit_K(3, 0),
            14: emit_T(3, 0),
            15: emit_K(3, 1),
            16: emit_T(3, 1),
        }

        # prologue: Q pass + first KV half of round 0
        for f in emit_A(0) + emit_K(0, 0):
            f()

        outq = deque()
        plist = {}
        ppvs = {}

        for g in range(len(PAIR_ORDER) + PDEPTH):
            if g < len(PAIR_ORDER):
                t, u = PAIR_ORDER[g]
                # scores pair: 2 k-blocks into one [128, 512] PSUM tile
                ps = psA.tile([128, PW], F32, tag="big", name=f"ps_{g}")
                for half in range(2):
                    kb = 2 * u + half
                    nc.tensor.matmul(
                        ps[:, half * QW:(half + 1) * QW],
                        kt_all[64:128, kb * 128:(kb + 1) * 128],
                        qt_all[64:128, t, :], start=True, stop=True)
                p_sb = ppool.tile([128, PW], BF16, tag="p", name=f"p_{g}")
                nc.scalar.activation(p_sb, ps, AF.Exp, scale=0.125)
                if u >= 2 * t:
                    nc.vector.tensor_mul(p_sb, p_sb, mask_pair_ap(u - 2 * t))
                plist[g] = p_sb
                for f in schedule.get(g, ()):  # chunk-gated filler
                    f()
                for _ in range(min(2, len(outq))):
                    outq.popleft()()
            j = g - PDEPTH
            if j >= 0:
                tj, uj = PAIR_ORDER[j]
                if uj == 0:
                    ppvs[tj] = psB.tile([H + 1, QW], F32, tag="p65",
                                        name=f"ppv_{tj}")
                last_u = LAST_PAIR[tj]
                for half in range(2):
                    kb = 2 * uj + half
                    nc.tensor.matmul(ppvs[tj], v_all[:, kb, :],
                                     plist[j][:, half * QW:(half + 1) * QW],
                                     start=(kb == 0),
                                     stop=(uj == last_u and half == 1))
                del plist[j]
                if uj == last_u:
                    outq.extend(emit_out(tj, ppvs[tj]))
        while outq:
            outq.popleft()()

    nc.compile()
    return nc


_NC_CACHE = None


def _get_nc():
    global _NC_CACHE
    if _NC_CACHE is None:
        _NC_CACHE = build_program()
    return _NC_CACHE


def make_host_inputs(x, Wq, bq, Wk, bk, Wv, bv):
    """Per-core input maps from the full problem inputs."""
    x = np.asarray(x, np.float32)
    wkv = np.hstack([np.asarray(Wv, np.float32), np.asarray(Wk, np.float32)])
    wkv_t = wkv.astype(NPBF16).reshape(NEC, 128, 128).transpose(1, 0, 2)
    wq_t = (np.asarray(Wq, np.float32).astype(NPBF16)
            .reshape(NEC, 128, H).transpose(1, 0, 2))
    identb = np.eye(128, dtype=NPBF16)

    # mask[p, j, f] = 1 iff query(256h + f) >= key(koff_h[j] + p), offsets
    # within the 512-tile in ORIGINAL order; device k-block 4t+j holds
    # original offset koff_h[j] after the per-core permutation.
    ff = np.arange(QW)[None, None, :]
    pp = np.arange(128)[:, None, None]
    wpacks = []
    for h in range(2):
        koff = np.array([0, 128, 256, 384] if h == 0 else [256, 384, 0, 128])
        m = ((256 * h + ff) >= (koff[None, :, None] + pp)).astype(NPBF16)
        bcols = np.zeros((128, 2), NPBF16)
        bcols[0:H, 0] = np.asarray(bv, np.float32).astype(NPBF16)
        bcols[H:128, 0] = np.asarray(bk, np.float32).astype(NPBF16)
        bcols[0:H, 1] = np.asarray(bq, np.float32).astype(NPBF16)
        wpacks.append(np.ascontiguousarray(np.concatenate(
            [wq_t.reshape(128, NEC * H), identb, bcols,
             wkv_t.reshape(128, NEC * 128), m.reshape(128, 4 * QW)],
            axis=1)))

    # x^T per (batch, half): device s-block g holds original block perm[g]
    maps = []
    for c in range(NCORES):
        b, h = c // 2, c % 2
        xtb = np.ascontiguousarray(x[b].astype(NPBF16).T)    # [E, S]
        if h == 1:
            blocks = xtb.reshape(E, S // 128, 128)
            # within each 512-tile: device [0,1,2,3] = orig [2,3,0,1]
            perm = np.arange(S // 128).reshape(-1, 4)[:, [2, 3, 0, 1]].ravel()
            xtb = np.ascontiguousarray(blocks[:, perm, :].reshape(E, S))
        xt_t = np.ascontiguousarray(
            xtb.reshape(NEC, 128, S).transpose(1, 0, 2))
        maps.append({"wpack": wpacks[h], "xt": xt_t})
    return maps


def assemble_output(results):
    """results: per-core {'y': [65, 4, 256]} keyed 0..7; host divides."""
    out = np.empty((B, S, H), np.float32)
    for c in range(NCORES):
        b, h = c // 2, c % 2
        y = np.asarray(results[c]["y"], np.float32)  # [65, t, f]
        o = (y[0:H] / y[H:H + 1]).transpose(1, 2, 0)  # [t, f, H]
        for t in range(NST):
            out[b, 512 * t + 256 * h: 512 * t + 256 * h + 256, :] = o[t]
    return out


def run_cores(in_maps, trace=False):
    from concourse.bass_utils import run_bass_kernel_spmd
    nc = _get_nc()
    return run_bass_kernel_spmd(nc, in_maps, list(range(NCORES)), trace=trace)


def kernel(x, Wq, bq, Wk, bk, Wv, bv):
    in_maps = make_host_inputs(x, Wq, bq, Wk, bk, Wv, bv)
    res = run_cores(in_maps).results
    return assemble_output(res)



# revision 8
# speedup vs baseline: 1.1116x; 1.1116x over previous
"""Trainium2 Bass kernel for single-head causal attention.

Problem: B=4, S=2048, E=1024, H=64 fp32.
  q = x@Wq+bq; k = x@Wk+bk; v = x@Wv+bv
  out = softmax(causal(q k^T / sqrt(H))) v

Sharding: 8 cores; core c = (batch b=c//2, query-half h=c%2).
Each core computes full K/V for its batch but attention for only its
1024 queries (h=0: front 256 of each 512-tile, h=1: back 256).
SPMD-uniform: the per-core query selection is realized by a host-side
128-column block permutation of x^T (h=1 swaps the half-tiles within
each 512 tile), and causality by per-core mask tables; the device
program is identical on all cores.

All matmul operands are bf16 (1 cycle/col on the PE vs 4 for fp32),
fp32 accumulation in PSUM. x is transposed and cast to bf16 on the
host: no PE transposes of x, and DMA traffic halves (4MB/core).
The device returns pv^T tiles [65, 256] per q-tile (row 64 = softmax
denominator from a ones-column in V); the host does the final
divide + transpose, removing the whole output transpose stage.

Engine assignment: PE = projections + scores + PV + V transposes
(plus a HAM-warmup matmul burst during the initial DMA wait);
ACT = exp (512-wide pairs) + Q copy; DVE = K/V copies, masks, V-block
copies, pv copies, bias cast. K^T/Q^T live on partitions 64:127
([Wv|Wk] packing) so every PSUM->SBUF copy is partition-aligned.
(gpsimd cannot touch PSUM on HW, so it only does memsets.)

The kernel is one software-pipelined stream over 20 score/PV "pairs"
(2 k-blocks each); rounds 2 and 3 are interleaved pair-by-pair so the
exp (ACT) load of round 3 overlaps PE-heavy projection work, and an
unmasked pair closes round 3 so the final exp->mask->PV tail is short.
Projection closures for future rounds (split into chunk-gated groups
A/K1/K2/T placed at slots matching the xt DMA arrival order) and
output stages are used as PE filler between pairs, so the in-order PE
queue never stalls on the exp->mask->PV chain (PDEPTH=2 lookahead).
PSUM banks (8): scores 2 + pkv 1 + ppv 2 + pq 1 + V-trans 2.
"""

import sys
from collections import deque
from contextlib import ExitStack

import numpy as np
import ml_dtypes

if "/opt/trn_rl_repo" not in sys.path:
    sys.path.insert(0, "/opt/trn_rl_repo")

import concourse.bacc as bacc
import concourse.mybir as mybir
import concourse.tile as tile

B, S, E, H = 4, 2048, 1024, 64
NCORES = 8
F32 = mybir.dt.float32
BF16 = mybir.dt.bfloat16
AF = mybir.ActivationFunctionType
NPBF16 = ml_dtypes.bfloat16

ST = 512          # s-tile width (projections / one round)
NST = S // ST     # 4 rounds
NEC = E // 128    # 8 e-chunks (contraction)
QW = 256          # per-core q-tile width in attention
PW = 2 * QW       # paired width (2 k-blocks per exp)
XCH = 256         # xt DMA chunk width
NCH = S // XCH    # 8 chunks
PDEPTH = 2        # attention software-pipeline depth, in pairs
NWARM = 5         # HAM-warmup matmuls (512 cols each)

# wpack column offsets (DMA-split points: wq+identb | wkv | mask)
W_Q = 0
W_ID = W_Q + NEC * H
W_BVK = W_ID + 128
W_BQ = W_BVK + 1
W_KV = W_BQ + 1
W_MASK = W_KV + NEC * 128
W_COLS = W_MASK + 4 * QW

# rounds 2/3 interleaved (round-3 exp load overlaps round-2+proj PE work),
# round 3 starting after its Q-pass chunk (x6); the final pairs are
# ordered so an unmasked pair closes round 3 (shorter tail chain).
PAIR_ORDER = [(0, 0), (0, 1),
              (1, 0), (1, 1), (1, 2), (1, 3),
              (2, 0), (2, 1), (2, 2), (3, 0), (2, 3), (3, 1), (2, 4),
              (3, 2), (2, 5), (3, 3), (3, 6), (3, 7), (3, 5), (3, 4)]
# last-emitted pair per round (closes that round's PV accumulation)
LAST_PAIR = {0: 1, 1: 3, 2: 5, 3: 4}


def build_program():
    nc = bacc.Bacc("TRN2", target_bir_lowering=False, debug=False,
                   num_devices=NCORES)

    wp_d = nc.dram_tensor("wpack", [128, W_COLS], BF16, kind="ExternalInput")
    xt_d = nc.dram_tensor("xt", [128, NCH, NEC, XCH], BF16,
                          kind="ExternalInput")
    y_d = nc.dram_tensor("y", [H + 1, NST, QW], F32, kind="ExternalOutput")

    with tile.TileContext(nc) as tc, ExitStack() as ctx:
        singles = ctx.enter_context(tc.tile_pool(name="singles", bufs=1))
        vtpool = ctx.enter_context(tc.tile_pool(name="vtpool", bufs=2))
        ppool = ctx.enter_context(tc.tile_pool(name="ppool", bufs=4))
        opool = ctx.enter_context(tc.tile_pool(name="opool", bufs=4))
        # PSUM 8 banks: sc-pairs 2 + kv 1 + ppv 2 + pq 1 + vtrans 2
        psA = ctx.enter_context(tc.tile_pool(name="psA", bufs=2, space="PSUM"))
        psKV = ctx.enter_context(tc.tile_pool(name="psKV", bufs=1,
                                              space="PSUM"))
        psB = ctx.enter_context(tc.tile_pool(name="psB", bufs=2, space="PSUM"))
        psQ = ctx.enter_context(tc.tile_pool(name="psQ", bufs=1, space="PSUM"))
        psC = ctx.enter_context(tc.tile_pool(name="psC", bufs=2, space="PSUM"))

        # ---- SBUF tiles ----
        wp = singles.tile([128, W_COLS], BF16)
        xt = singles.tile([128, NCH, NEC, XCH], BF16)

        # ---- DMAs, in dependency-priority order ----
        # xt chunks are contiguous 4 KiB/partition runs (host packs
        # [128, chunk, ec, col]) so the SDMA engines hit line rate.
        def xt_dma(xc):
            nc.sync.dma_start(out=xt[:, xc], in_=xt_d[:, xc])

        nc.sync.dma_start(out=wp[:, W_Q:W_KV], in_=wp_d[:, W_Q:W_KV])
        xt_dma(0)
        nc.sync.dma_start(out=wp[:, W_KV:W_MASK], in_=wp_d[:, W_KV:W_MASK])
        xt_dma(1)
        xt_dma(2)
        nc.sync.dma_start(out=wp[:, W_MASK:W_MASK + PW],
                          in_=wp_d[:, W_MASK:W_MASK + PW])
        xt_dma(3)
        nc.sync.dma_start(out=wp[:, W_MASK + PW:W_COLS],
                          in_=wp_d[:, W_MASK + PW:W_COLS])
        xt_dma(4)
        xt_dma(6)   # round-3 Q-pass chunk before the round-2/3 k-tails
        xt_dma(5)
        xt_dma(7)

        def wkv_ap(ec):   # [Wv | Wk] chunk: out rows 0:64 = V, 64:128 = K
            return wp[:, W_KV + ec * 128: W_KV + (ec + 1) * 128]

        def wq_ap(ec):
            return wp[:, W_Q + ec * H: W_Q + (ec + 1) * H]

        def mask_pair_ap(jj):   # [128, 512] = k-blocks (2jj, 2jj+1)
            return wp[:, W_MASK + jj * PW: W_MASK + (jj + 1) * PW]

        identb = wp[:, W_ID:W_ID + 128]
        # HAM warmup: dummy matmuls on a just-memset tile run while the
        # first DMAs stream, so the PE clock-gate is already released
        # (and the sim's p-state ramp elapsed) when real work arrives.
        # 512-col matmuls keep the PE busy window unbroken until the
        # first xt chunk lands (~2.5 us after the PE clears its
        # preamble).
        warm = singles.tile([128, 512], BF16)
        nc.vector.memset(warm, 0.0)
        for w in range(NWARM):
            pw_ = psA.tile([128, PW], F32, tag="big", name=f"warm_{w}")
            nc.tensor.matmul(pw_, warm[:, 0:128], warm,
                             start=True, stop=True)
        # biases ride in wpack as bf16; engines need fp32 scalars
        fb = singles.tile([128, 2], F32)
        nc.vector.tensor_copy(fb, wp[:, W_BVK:W_BVK + 2])
        bvk = fb[:, 0:1]        # rows 0:64 = bv, rows 64:128 = bk
        bq = fb[0:H, 1:2]

        qt_all = singles.tile([128, NST, QW], BF16)  # rows 64:128 = own Q^T
        kt_all = singles.tile([128, S], BF16)        # rows 64:128 = K^T
        v_all = singles.tile([128, S // 128, H + 1], BF16)  # V blocks
        nc.vector.memset(v_all[:, :, H:H + 1], 1.0)

        # ---- emission helpers ----
        # Projection work for round t is split into chunk-gated groups so
        # the in-order PE queue never blocks on a not-yet-arrived xt chunk:
        #   A(t)  = Q pass + qt copy            (needs xt chunk 2t)
        #   K1(t) = KV pass cols 0:256 + copies (needs xt chunk 2t)
        #   K2(t) = KV pass cols 256:512        (needs xt chunk 2t+1)
        #   T1/T2(t) = V-block transposes for each half
        proj_states = {}

        def emit_A(t):
            def f():
                st_ = proj_states.setdefault(t, {})
                st_["pq"] = psQ.tile([H, QW], F32, tag="pq",
                                     name=f"pq_{t}")
                for ec in range(NEC):
                    nc.tensor.matmul(st_["pq"], wq_ap(ec),
                                     xt[:, 2 * t, ec, :],
                                     start=(ec == 0), stop=(ec == NEC - 1))
                nc.scalar.activation(qt_all[64:128, t, :], st_["pq"],
                                     AF.Identity, bias=bq)
            return [f]

        def emit_K(t, s):
            c0, c1 = s * QW, (s + 1) * QW
            def f():
                st_ = proj_states.setdefault(t, {})
                if s == 0:
                    st_["pkv"] = psKV.tile([128, ST], F32, tag="kv",
                                           name=f"pkv_{t}")
                    st_["vt"] = vtpool.tile([H, ST], BF16, tag="vt",
                                            name=f"vt_{t}")
                pkv = st_["pkv"]
                for ec in range(NEC):
                    nc.tensor.matmul(pkv[:, c0:c1], wkv_ap(ec),
                                     xt[:, 2 * t + s, ec, :],
                                     start=(ec == 0), stop=(ec == NEC - 1))
                nc.vector.tensor_scalar_add(st_["vt"][:, c0:c1],
                                            pkv[0:H, c0:c1], bvk[0:H, :])
                nc.vector.tensor_scalar_add(
                    kt_all[64:128, t * ST + c0:t * ST + c1],
                    pkv[64:128, c0:c1], bvk[64:128, :])
            return [f]

        def emit_T(t, s):
            def f():
                vt = proj_states[t]["vt"]
                pv = psC.tile([128, 2, H], BF16, tag="small",
                              name=f"pvt_{t}_{s}")
                for i in range(2):
                    sb = 2 * s + i
                    nc.tensor.transpose(pv[:, i, :],
                                        vt[:, sb * 128:(sb + 1) * 128],
                                        identb[0:H, 0:H])
                blk = t * 4 + 2 * s
                nc.vector.tensor_copy(v_all[:, blk:blk + 2, 0:H], pv)
            return [f]

        def emit_out(t, ppv):
            """Output closures for round t: copy pv^T out; host divides."""
            state = {}

            def copy():
                state["pv_sb"] = opool.tile([H + 1, QW], F32, tag="pv_sb",
                                            name=f"pvsb_{t}")
                nc.vector.tensor_copy(state["pv_sb"], ppv)

            def dma():
                nc.sync.dma_start(out=y_d[:, t, :], in_=state["pv_sb"])

            return [copy, dma]

        # ---- static filler schedule (slot -> closure groups) ----
        # Placement tracks the xt DMA arrival order:
        # x0,[w],x1,x2,[mask01],x3,[mask23],x4,x6,x5,x7
        schedule = {
            0: emit_K(0, 1) + emit_T(0, 0),
            1: emit_T(0, 1) + emit_A(1),
            2: emit_K(1, 0),
            3: emit_T(1, 0),
            4: emit_K(1, 1) + emit_T(1, 1),
            5: emit_A(2),
            6: emit_K(2, 0),
            7: emit_T(2, 0) + emit_A(3),
            10: emit_K(2, 1),
            11: emit_T(2, 1),
            12: emit_K(3, 0),
            14: emit_T(3, 0),
            15: emit_K(3, 1),
            16: emit_T(3, 1),
        }

        # prologue: Q pass + first KV half of round 0
        for f in emit_A(0) + emit_K(0, 0):
            f()

        outq = deque()
        plist = {}
        ppvs = {}

        for g in range(len(PAIR_ORDER) + PDEPTH):
            if g < len(PAIR_ORDER):
                t, u = PAIR_ORDER[g]
                # scores pair: 2 k-blocks into one [128, 512] PSUM tile
                ps = psA.tile([128, PW], F32, tag="big", name=f"ps_{g}")
                for half in range(2):
                    kb = 2 * u + half
                    nc.tensor.matmul(
                        ps[:, half * QW:(half + 1) * QW],
                        kt_all[64:128, kb * 128:(kb + 1) * 128],
                        qt_all[64:128, t, :], start=True, stop=True)
                p_sb = ppool.tile([128, PW], BF16, tag="p", name=f"p_{g}")
                nc.scalar.activation(p_sb, ps, AF.Exp, scale=0.125)
                if u >= 2 * t:
                    nc.vector.tensor_mul(p_sb, p_sb, mask_pair_ap(u - 2 * t))
                plist[g] = p_sb
                for f in schedule.get(g, ()):  # chunk-gated filler
                    f()
                for _ in range(min(2, len(outq))):
                    outq.popleft()()
            j = g - PDEPTH
            if j >= 0:
                tj, uj = PAIR_ORDER[j]
                if uj == 0:
                    ppvs[tj] = psB.tile([H + 1, QW], F32, tag="p65",
                                        name=f"ppv_{tj}")
                last_u = LAST_PAIR[tj]
                for half in range(2):
                    kb = 2 * uj + half
                    nc.tensor.matmul(ppvs[tj], v_all[:, kb, :],
                                     plist[j][:, half * QW:(half + 1) * QW],
                                     start=(kb == 0),
                                     stop=(uj == last_u and half == 1))
                del plist[j]
                if uj == last_u:
                    outq.extend(emit_out(tj, ppvs[tj]))
        while outq:
            outq.popleft()()

    nc.compile()
    return nc


_NC_CACHE = None


def _get_nc():
    global _NC_CACHE
    if _NC_CACHE is None:
        _NC_CACHE = build_program()
    return _NC_CACHE


def make_host_inputs(x, Wq, bq, Wk, bk, Wv, bv):
    """Per-core input maps from the full problem inputs."""
    x = np.asarray(x, np.float32)
    wkv = np.hstack([np.asarray(Wv, np.float32), np.asarray(Wk, np.float32)])
    wkv_t = wkv.astype(NPBF16).reshape(NEC, 128, 128).transpose(1, 0, 2)
    wq_t = (np.asarray(Wq, np.float32).astype(NPBF16)
            .reshape(NEC, 128, H).transpose(1, 0, 2))
    identb = np.eye(128, dtype=NPBF16)

    # mask[p, j, f] = 1 iff query(256h + f) >= key(koff_h[j] + p), offsets
    # within the 512-tile in ORIGINAL order; device k-block 4t+j holds
    # original offset koff_h[j] after the per-core permutation.
    ff = np.arange(QW)[None, None, :]
    pp = np.arange(128)[:, None, None]
    wpacks = []
    for h in range(2):
        koff = np.array([0, 128, 256, 384] if h == 0 else [256, 384, 0, 128])
        m = ((256 * h + ff) >= (koff[None, :, None] + pp)).astype(NPBF16)
        bcols = np.zeros((128, 2), NPBF16)
        bcols[0:H, 0] = np.asarray(bv, np.float32).astype(NPBF16)
        bcols[H:128, 0] = np.asarray(bk, np.float32).astype(NPBF16)
        bcols[0:H, 1] = np.asarray(bq, np.float32).astype(NPBF16)
        wpacks.append(np.ascontiguousarray(np.concatenate(
            [wq_t.reshape(128, NEC * H), identb, bcols,
             wkv_t.reshape(128, NEC * 128), m.reshape(128, 4 * QW)],
            axis=1)))

    # x^T per (batch, half): device s-block g holds original block perm[g]
    maps = []
    for c in range(NCORES):
        b, h = c // 2, c % 2
        xtb = np.ascontiguousarray(x[b].astype(NPBF16).T)    # [E, S]
        if h == 1:
            blocks = xtb.reshape(E, S // 128, 128)
            # within each 512-tile: device [0,1,2,3] = orig [2,3,0,1]
            perm = np.arange(S // 128).reshape(-1, 4)[:, [2, 3, 0, 1]].ravel()
            xtb = np.ascontiguousarray(blocks[:, perm, :].reshape(E, S))
        # [128, chunk, ec, col]: each chunk is a contiguous 4 KiB
        # per-partition run (DMA line rate); matmuls read
        # xt[:, chunk, ec, :] slabs.
        xt_t = np.ascontiguousarray(
            xtb.reshape(NEC, 128, NCH, XCH).transpose(1, 2, 0, 3))
        maps.append({"wpack": wpacks[h], "xt": xt_t})
    return maps


def assemble_output(results):
    """results: per-core {'y': [65, 4, 256]} keyed 0..7; host divides."""
    out = np.empty((B, S, H), np.float32)
    for c in range(NCORES):
        b, h = c // 2, c % 2
        y = np.asarray(results[c]["y"], np.float32)  # [65, t, f]
        o = (y[0:H] / y[H:H + 1]).transpose(1, 2, 0)  # [t, f, H]
        for t in range(NST):
            out[b, 512 * t + 256 * h: 512 * t + 256 * h + 256, :] = o[t]
    return out


def run_cores(in_maps, trace=False):
    from concourse.bass_utils import run_bass_kernel_spmd
    nc = _get_nc()
    return run_bass_kernel_spmd(nc, in_maps, list(range(NCORES)), trace=trace)


def kernel(x, Wq, bq, Wk, bk, Wv, bv):
    in_maps = make_host_inputs(x, Wq, bq, Wk, bk, Wv, bv)
    res = run_cores(in_maps).results
    return assemble_output(res)

